# revision 27
# baseline (speedup 1.0000x reference)
"""GAT (3-layer, 4-head) graph-classification kernel for 8 Trainium2 NeuronCores.

Strategy (dst-sharded message passing, super-row gathers):
  - Nodes are degree-sorted and dealt round-robin to 8 cores (graph/data
    parallel); each core's nodes are laid out tile-major (49 tiles x 128).
  - Per layer: each core computes h|al_src|al_dst for its node shard with one
    matmul (x_T @ [W | W@Asrc | W@Adst]), writes packed bf16 640B rows
    (h[256] | al_src f32 | pad) to a local HBM table shard, then an 8-core
    AllGather replicates the full node table.
  - Edges are sharded by destination. Per-edge source rows are fetched with
    GPSIMD dma_gather at SUPER-row granularity: one 1280B descriptor covers a
    PAIR of adjacent table rows, halving descriptor count and keeping int16
    indices in range (25088 supers < 32767) with a single region. The
    wrong-half ("phantom") slot of each pair is killed via the attention mask
    (-inf logit => alpha=0). Gather descriptors round-robin 4 SWDGE queues.
  - Attention softmax per destination runs on VectorE/ScalarE over 2x virtual
    slots; messages are alpha-weighted in place and segment-summed along the
    free dimension.
  - Layer outputs are transposed back to feature-major (TensorE) to feed the
    next layer's matmul; after layer 3 a one-hot matmul pools node features
    into per-graph sums. Host sums the 8 per-core partial graph outputs.
"""

import sys

for _p in ("/opt/trn_rl_repo",):
    if _p not in sys.path:
        sys.path.insert(0, _p)

import numpy as np
import ml_dtypes

import concourse.bass as bass
import concourse.bacc as bacc
import concourse.mybir as mybir
import concourse.tile as tile
from concourse import library_config
from concourse.bass_utils import run_bass_kernel_spmd

FP = mybir.dt.float32
BF = mybir.dt.bfloat16
I16 = mybir.dt.int16
BFNP = ml_dtypes.bfloat16

# Problem constants (hardcoded per the harness contract).
N = 50000
E = 800000
IN = 128
H = 4
D = 64
HD = 256
G = 64
NEG = 0.2

NCORES = 8
TILES = 49                 # 128-node tiles per core
SHARD = TILES * 128        # 6272 rows per core (6250 real + 22 pad)
TOTROWS = NCORES * SHARD   # 50176
NSUP = TOTROWS // 2        # 25088 super-rows (fits int16)
ROWW = 320                 # bf16 columns per table row (640 B): h[256] | al_src f32[4] | pad
SUPW = 2 * ROWW            # 640 cols = 1280 B per gather descriptor
CMAX = 36                  # max super-slot columns per tile-group
TMAX = 6                   # max 128-dst tiles per group
QB = 4                     # phase-A chunks per staging DMA
NQUEUES = 4                # SWDGE queues for gather round-robin
AGCH = 4                   # AllGather chunks per layer (overlap with phase A)
NEGINF = -1.0e30

_cache = {}


# ----------------------------------------------------------------------------
# Host-side preprocessing
# ----------------------------------------------------------------------------

def _preprocess(edge_index, batch, sort_slots=True):
    src = np.concatenate([edge_index[0], np.arange(N, dtype=np.int64)])
    dst = np.concatenate([edge_index[1], np.arange(N, dtype=np.int64)])
    deg = np.bincount(dst, minlength=N)

    # deal nodes to cores by degree rank (load balance + uniform tile widths)
    order = np.argsort(-deg, kind="stable")
    core_nodes = np.full((NCORES, TILES * 128), -1, np.int64)
    node2row = np.full(N, -1, np.int64)
    for c in range(NCORES):
        nodes = order[c::NCORES]
        core_nodes[c, : len(nodes)] = nodes  # index = t*128 + p (tile-major)
        node2row[nodes] = c * SHARD + np.arange(len(nodes))

    # CSR by destination
    eorder = np.argsort(dst, kind="stable")
    row_by = node2row[src][eorder]
    dst_by = dst[eorder]
    starts = np.searchsorted(dst_by, np.arange(N))
    n_edges = E + N

    # per-tile slot widths, shared across cores for SPMD
    L = np.zeros(TILES, np.int64)
    for t in range(TILES):
        nodes_t = core_nodes[:, t * 128 : (t + 1) * 128].reshape(-1)
        real = nodes_t >= 0
        if real.any():
            L[t] = deg[nodes_t[real]].max()

    def pad4(v):
        return (int(v) + 3) // 4 * 4

    groups = []  # (t0, T, gL) with gL padded to a multiple of 4
    t = 0
    while t < TILES:
        T = 1
        while (
            T < TMAX
            and t + T < TILES
            and (T + 1) * pad4(max(int(L[t : t + T].max()), int(L[t + T]))) <= CMAX
        ):
            T += 1
        groups.append((t, T, pad4(L[t : t + T].max())))
        t += T

    tot_slots = sum(T * 128 * gL for (_, T, gL) in groups)

    # per-core packed idx / mask arrays
    XI = sum(T * gL * 8 for (_, T, gL) in groups)
    XM = sum(2 * T * gL for (_, T, gL) in groups)
    idx_all = np.zeros((NCORES, 128, XI), np.int16)
    mask_all = np.full((NCORES, 128, XM), NEGINF, np.float32)
    goffs = []  # (idx col off, mask col off) per group

    for c in range(NCORES):
        io = 0
        mo = 0
        for gi, (t0, T, gL) in enumerate(groups):
            if c == 0:
                goffs.append((io, mo))
            C = T * gL
            blk = np.zeros((C, 128), np.int16)
            for ti in range(T):
                nodes_t = core_nodes[c, (t0 + ti) * 128 : (t0 + ti + 1) * 128]
                safe = np.maximum(nodes_t, 0)
                dd = np.where(nodes_t >= 0, deg[safe], 0)
                st = starts[safe]
                ji = st[:, None] + np.arange(gL)[None, :]
                rows = row_by[np.minimum(ji, n_edges - 1)]
                valid = np.arange(gL)[None, :] < dd[:, None]
                if sort_slots:
                    # sort each dst's slot list by row id: descriptor streams
                    # then walk the table roughly in order (HBM locality)
                    rows = np.where(valid, rows, 1 << 30)
                    rows = np.sort(rows, axis=1)
                # pad slots are masked out; spread their dummy reads across
                # the table to avoid hammering one HBM row
                spread = (
                    np.arange(128)[:, None] * 97 + np.arange(gL)[None, :] * 3181
                ) % NSUP
                rows = np.where(valid, rows, spread * 2)
                sup = rows >> 1
                par = rows & 1
                blk[ti * gL : (ti + 1) * gL, :] = sup.T.astype(np.int16)
                # mask per virtual column: 2*(ti*gL + j) + half
                mslice = np.full((128, gL, 2), NEGINF, np.float32)
                okh = np.stack([(par == 0) & valid, (par == 1) & valid], axis=-1)
                mslice[okh] = 0.0
                mask_all[
                    c, :, mo + 2 * ti * gL : mo + 2 * (ti + 1) * gL
                ] = mslice.reshape(128, 2 * gL)
            w = blk.reshape(-1).reshape(-1, 16).T  # [16, C*8]
            idx_all[c, :, io : io + C * 8] = np.tile(w, (8, 1))
            io += C * 8
            mo += 2 * C
        assert io == XI and mo == XM

    # pooling one-hot [p, t*G + g]
    onehot = np.zeros((NCORES, 128, TILES * G), np.float32)
    for c in range(NCORES):
        nodes = core_nodes[c]
        real = nodes >= 0
        tt = np.arange(TILES * 128) // 128
        pp = np.arange(TILES * 128) % 128
        gid = batch[np.maximum(nodes, 0)]
        onehot[c, pp[real], tt[real] * G + gid[real]] = 1.0

    return dict(
        core_nodes=core_nodes,
        groups=groups,
        goffs=goffs,
        idx_all=idx_all,
        mask_all=mask_all,
        onehot=onehot,
        XI=XI,
        XM=XM,
        tot_slots=tot_slots,
    )


def _build_wcat(W, a_src, a_dst):
    F = W.shape[0]
    Asrc = np.zeros((HD, H), np.float64)
    Adst = np.zeros((HD, H), np.float64)
    for h in range(H):
        Asrc[h * D : (h + 1) * D, h] = a_src[h]
        Adst[h * D : (h + 1) * D, h] = a_dst[h]
    Wc = np.zeros((F, 264), np.float64)
    Wc[:, 0:256] = W
    Wc[:, 256:260] = W @ Asrc
    Wc[:, 260:264] = W @ Adst
    return Wc.astype(BFNP)


# ----------------------------------------------------------------------------
# Bass program
# ----------------------------------------------------------------------------

def _build_program(meta, stage=3, repeat=1, nqueues=NQUEUES):
    groups = meta["groups"]
    goffs = meta["goffs"]
    XI, XM = meta["XI"], meta["XM"]
    CSMAX = max(T * gL for (_, T, gL) in groups)       # super-slot columns
    CVMAX = 2 * CSMAX                                  # virtual slot columns
    TMAXG = max(T for (_, T, _) in groups)

    nc = bacc.Bacc(
        "TRN2",
        target_bir_lowering=False,
        debug=False,
        enable_asserts=False,
        num_devices=NCORES,
        num_swdge_queues=nqueues,
    )

    d_x0T = nc.dram_tensor("x0T", [IN, SHARD], BF, kind="ExternalInput")
    d_wcat = [
        nc.dram_tensor(f"wcat{l}", [128 if l == 0 else 256, 264], BF, kind="ExternalInput")
        for l in range(3)
    ]
    d_bias = [
        nc.dram_tensor(f"bias{l}", [128, 256], FP, kind="ExternalInput") for l in range(3)
    ]
    d_ident = nc.dram_tensor("ident", [128, 128], BF, kind="ExternalInput")
    d_idx = nc.dram_tensor("idxall", [128, XI], I16, kind="ExternalInput")
    d_mask = nc.dram_tensor("maskall", [128, XM], BF, kind="ExternalInput")
    d_onehot = nc.dram_tensor("onehot", [128, TILES * G], BF, kind="ExternalInput")
    d_out = nc.dram_tensor("pooled", [G, HD], FP, kind="ExternalOutput")

    with tile.TileContext(nc) as tc:
        nc.gpsimd.load_library(library_config.mlp)
        with (
            tc.tile_pool(name="const", bufs=1) as cpool,
            tc.tile_pool(name="gath", bufs=3) as gpool,
            tc.tile_pool(name="att", bufs=2) as epool,
            tc.tile_pool(name="stage", bufs=2) as spool,
            tc.tile_pool(name="og", bufs=2) as ogpool,
            tc.tile_pool(name="oh", bufs=2) as ohpool,
            tc.tile_pool(name="psA", bufs=2, space="PSUM") as pspool,
            tc.tile_pool(name="psT", bufs=2, space="PSUM") as pstp,
            tc.tile_pool(name="psP", bufs=1, space="PSUM") as ppool,
            tc.tile_pool(name="dram", bufs=1, space="DRAM") as dpool,
        ):
            # resident tiles
            xT_a = cpool.tile([128, SHARD], BF, tag="xTa")
            xT_b = cpool.tile([128, SHARD], BF, tag="xTb")
            wcat_sb = []
            for l in range(3):
                ks = 1 if l == 0 else 2
                tiles_l = [
                    cpool.tile([128, 264], BF, name=f"wc{l}{k}", tag=f"wc{l}{k}")
                    for k in range(ks)
                ]
                wcat_sb.append(tiles_l)
            bias_sb = [cpool.tile([128, 256], FP, name=f"b{l}", tag=f"b{l}") for l in range(3)]
            ident = cpool.tile([128, 128], BF, tag="ident")
            idx_sb = cpool.tile([128, XI], I16, tag="idx")
            mask_sb = cpool.tile([128, XM], BF, tag="mask")
            aldst = cpool.tile([128, TILES * 4], FP, tag="aldst")

            tableshards = [
                dpool.tile(
                    [SHARD, ROWW], BF, name=f"tshard{lr}", tag=f"tshard{lr}"
                )
                for lr in range(3 * repeat)
            ]
            tablefulls = [
                dpool.tile(
                    [TOTROWS, ROWW],
                    BF,
                    name=f"tfull{lr}",
                    tag=f"tfull{lr}",
                    addr_space="Shared",
                )
                for lr in range(3 * repeat)
            ]

            # constant loads
            nc.sync.dma_start(xT_a[:], d_x0T[:])
            for l in range(3):
                for k, wt in enumerate(wcat_sb[l]):
                    nc.sync.dma_start(wt[:], d_wcat[l][k * 128 : (k + 1) * 128, :])
                nc.sync.dma_start(bias_sb[l][:], d_bias[l][:])
            nc.sync.dma_start(ident[:], d_ident[:])
            nc.sync.dma_start(idx_sb[:], d_idx[:])
            nc.sync.dma_start(mask_sb[:], d_mask[:])

            nlayers = 2 if stage == 2 else (3 if stage == 3 else 1)
            for rep in range(repeat):
              pool_ps = (
                ppool.tile([64, 256], FP, name="pool_ps", tag="poolps")
                if stage == 3
                else None
              )
              for l in range(nlayers):
                ks = 1 if l == 0 else 2
                tableshard = tableshards[rep * 3 + l]
                tablefull = tablefulls[rep * 3 + l]
                tsh3 = tableshard.rearrange("(q p) w -> p q w", p=128)
                tabsup = tablefull.rearrange("(a b) w -> a (b w)", b=2)
                # ---- phase A: node transform + table shard. high_priority
                # per chunk interleaves this work with the PREVIOUS layer's
                # edge phase in the Tile schedule (deps allow it: chunk q only
                # needs the transposes of the edge group containing tile q).
                for q0 in range(0, TILES, QB):
                    nq = min(QB, TILES - q0)
                    hp = tc.high_priority(
                        offset=(TILES - q0) * 17 if (l > 0 or rep > 0) else 0
                    )
                    hp.__enter__()
                    stg = spool.tile([128, QB * ROWW], BF, tag="stg")
                    stg3 = stg[:].rearrange("p (q w) -> p q w", w=ROWW)
                    stgf = stg[:].bitcast(FP).rearrange("p (q w) -> p q w", w=ROWW // 2)
                    nc.vector.memset(stg3[:, :, 264:ROWW], 0)
                    for qi in range(nq):
                        q = q0 + qi
                        ps = pspool.tile([128, 264], FP, tag="psA")
                        nc.tensor.matmul(
                            ps[:],
                            xT_a[:, q * 128 : (q + 1) * 128],
                            wcat_sb[l][0][:],
                            start=True,
                            stop=(ks == 1),
                        )
                        if ks == 2:
                            nc.tensor.matmul(
                                ps[:],
                                xT_b[:, q * 128 : (q + 1) * 128],
                                wcat_sb[l][1][:],
                                start=False,
                                stop=True,
                            )
                        nc.scalar.copy(stg3[:, qi, 0:256], ps[:, 0:256])
                        nc.vector.tensor_copy(stgf[:, qi, 128:132], ps[:, 256:260])
                        nc.vector.tensor_copy(
                            aldst[:, q * 4 : (q + 1) * 4], ps[:, 260:264]
                        )
                    nc.sync.dma_start(
                        tsh3[:, q0 : q0 + nq, :], stg3[:, 0:nq, :]
                    )
                    hp.__exit__(None, None, None)

                # ---------------- allgather the packed node table ----------
                nc.gpsimd.collective_compute(
                    "AllGather",
                    mybir.AluOpType.bypass,
                    replica_groups=[list(range(NCORES))],
                    ins=[tableshard.opt()],
                    outs=[tablefull.opt()],
                )

                # ---------------- edge phase -------------------------------
                if stage == 0:
                    continue
                estage = stage if stage >= 10 else 99
                qctr = [0]
                for gi, (t0, T, gL) in enumerate(groups):
                    io, mo = goffs[gi]
                    C = T * gL       # super slots
                    CV = 2 * C       # virtual slots
                    jL = 2 * gL      # virtual slots per tile

                    hx = gpool.tile([128, CSMAX * SUPW], BF, tag="hx")
                    hxs = hx[:].rearrange("p (c w) -> p c w", w=SUPW)
                    hx3 = hx[:].rearrange("p (c w) -> p c w", w=ROWW)

                    # device limit: <=1024 indices per dma_gather instruction.
                    # high_priority hoists gathers ~2 groups earlier in the
                    # Tile list schedule so desc-gen + DMA overlap the previous
                    # groups' vector work instead of serializing behind it.
                    with tc.high_priority(offset=70):
                        for k0 in range(0, C, 8):
                            kc = min(8, C - k0)
                            nc.gpsimd.dma_gather(
                                hxs[:, k0 : k0 + kc, :],
                                tabsup[0:NSUP, 0:SUPW],
                                idx_sb[:, io + k0 * 8 : io + (k0 + kc) * 8],
                                kc * 128,
                                kc * 128,
                                SUPW,
                                queue_num=qctr[0] % nqueues,
                            )
                            qctr[0] += 1

                    e = epool.tile([128, CVMAX * 4], FP, tag="e")
                    if estage == 10:
                        nc.vector.reduce_max(
                            e[:, 0:1],
                            hx3[:, 0:CV, 0:1].rearrange("p c o -> p o c"),
                            axis=mybir.AxisListType.X,
                        )
                        continue

                    hxf = hx[:].bitcast(FP).rearrange("p (c w) -> p c w", w=ROWW // 2)
                    # alS[p, cv, h] at f32 columns 128..132 of each row
                    e3 = e[:].rearrange("p (c h) -> p c h", h=4)
                    ab = epool.tile([128, CVMAX * 4], BF, tag="ab")
                    ab3 = ab[:].rearrange("p (c h) -> p c h", h=4)

                    alD = aldst[:].rearrange("p (t h) -> p t h", h=4)[
                        :, t0 : t0 + T, :
                    ]

                    # logits: e = al_src[src] + al_dst[dst]
                    alS_r = hxf[:, 0:CV, 128:132].rearrange(
                        "p (t j) h -> p t j h", j=jL
                    )
                    alD_b = alD.unsqueeze(2).broadcast_to((128, T, jL, 4))
                    e4 = e3[:, 0:CV, :].rearrange("p (t j) h -> p t j h", j=jL)
                    nc.vector.tensor_add(e4, alS_r, alD_b)

                    eflat = e[:, : CV * 4]
                    # leaky relu in one op: e = max(NEG*e, e)  (valid for NEG<1)
                    nc.vector.scalar_tensor_tensor(
                        eflat,
                        eflat,
                        NEG,
                        eflat,
                        op0=mybir.AluOpType.mult,
                        op1=mybir.AluOpType.max,
                    )
                    mask_b = (
                        mask_sb[:, mo : mo + CV].unsqueeze(2).broadcast_to((128, CV, 4))
                    )
                    nc.vector.tensor_add(e3[:, 0:CV, :], e3[:, 0:CV, :], mask_b)

                    # segment max over virtual slots of each tile
                    m = epool.tile([128, TMAXG * 4], FP, name="m", tag="m")
                    in_m = e3[:, 0:CV, :].rearrange("p (t j) h -> p t h j", j=jL)
                    nc.vector.reduce_max(
                        m[:, : T * 4], in_m, axis=mybir.AxisListType.X
                    )
                    m3 = m[:].rearrange("p (t h) -> p t h", h=4)[:, 0:T, :]

                    # ex = exp(e - m)
                    m_b = m3.unsqueeze(2).broadcast_to((128, T, jL, 4))
                    nc.vector.tensor_sub(e4, e4, m_b)
                    nc.scalar.activation(
                        eflat, eflat, mybir.ActivationFunctionType.Exp
                    )

                    # denom and reciprocal
                    den = epool.tile([128, TMAXG * 4], FP, name="den", tag="den")
                    nc.vector.reduce_sum(
                        den[:, : T * 4],
                        e3[:, 0:CV, :].rearrange("p (t j) h -> p t h j", j=jL),
                        axis=mybir.AxisListType.X,
                    )
                    rec = epool.tile([128, TMAXG * 4], FP, tag="rec")
                    nc.vector.reciprocal(rec[:, : T * 4], den[:, : T * 4])
                    r3 = rec[:].rearrange("p (t h) -> p t h", h=4)[:, 0:T, :]

                    # alpha = ex / denom, cast to bf16
                    r_b = r3.unsqueeze(2).broadcast_to((128, T, jL, 4))
                    ab4 = ab3[:, 0:CV, :].rearrange("p (t j) h -> p t j h", j=jL)
                    nc.vector.tensor_mul(ab4, e4, r_b)

                    if estage == 11:
                        continue
                    # messages: h *= alpha, even/odd halves as 3D packed-f views
                    for off, par in ((0, 0), (ROWW, 1)):
                        hv = hxs[:, 0:C, off : off + 256].rearrange(
                            "p c (h d) -> p c h d", d=D
                        )
                        av = (
                            ab3[:, par : CV : 2, :]
                            .unsqueeze(3)
                            .broadcast_to((128, C, 4, D))
                        )
                        nc.vector.tensor_mul(hv, hv, av)

                    # segment sum via in-place pair-add tree (gL % 4 == 0):
                    #   L1: even half += odd half (within super)
                    #   L2: super pairs, L3: super quads, then reduce gL/4 roots
                    hxt = hx[:, : C * SUPW].rearrange(
                        "p (t g w) -> p t g w", g=gL, w=SUPW
                    )
                    nc.vector.tensor_add(
                        hxt[:, :, :, 0:256],
                        hxt[:, :, :, 0:256],
                        hxt[:, :, :, ROWW : ROWW + 256],
                    )
                    hx8 = hx[:, : C * SUPW].rearrange(
                        "p (t q x) -> p t q x", q=gL // 2, x=2 * SUPW
                    )
                    nc.vector.tensor_add(
                        hx8[:, :, :, 0:256],
                        hx8[:, :, :, 0:256],
                        hx8[:, :, :, SUPW : SUPW + 256],
                    )
                    hx16 = hx[:, : C * SUPW].rearrange(
                        "p (t q x) -> p t q x", q=gL // 4, x=4 * SUPW
                    )
                    nc.vector.tensor_add(
                        hx16[:, :, :, 0:256],
                        hx16[:, :, :, 0:256],
                        hx16[:, :, :, 2 * SUPW : 2 * SUPW + 256],
                    )
                    og = ogpool.tile([128, TMAXG * 256], FP, tag="og")
                    nc.vector.reduce_sum(
                        og[:, : T * 256],
                        hx16[:, :, :, 0:256].rearrange("p t q f -> p t f q"),
                        axis=mybir.AxisListType.X,
                    )

                    # bias + relu
                    og3 = og[:].rearrange("p (t f) -> p t f", f=256)
                    bias_b = bias_sb[l][:].unsqueeze(1).broadcast_to((128, T, 256))
                    nc.vector.tensor_add(og3[:, 0:T, :], og3[:, 0:T, :], bias_b)
                    relu_b = ogpool.tile([128, TMAXG * 256], BF, tag="relub")
                    nc.scalar.activation(
                        relu_b[:, : T * 256],
                        og[:, : T * 256],
                        mybir.ActivationFunctionType.Relu,
                    )

                    if estage == 12:
                        continue
                    if l < 2:
                        rb3 = relu_b[:].rearrange("p (t f) -> p t f", f=256)
                        for ti in range(T):
                            for fb, xt in ((0, xT_a), (1, xT_b)):
                                pt = pstp.tile([128, 128], BF, tag="psT")
                                nc.tensor.transpose(
                                    pt[:],
                                    rb3[:, ti, fb * 128 : (fb + 1) * 128],
                                    ident[:],
                                )
                                nc.scalar.copy(
                                    xt[:, (t0 + ti) * 128 : (t0 + ti + 1) * 128],
                                    pt[:],
                                )
                    else:
                        oh = ohpool.tile([128, TMAXG * G], BF, tag="oh")
                        nc.sync.dma_start(
                            oh[:, : T * G], d_onehot[:, t0 * G : (t0 + T) * G]
                        )
                        rb3p = relu_b[:].rearrange("p (t f) -> p t f", f=256)
                        for ti in range(T):
                            q = t0 + ti
                            nc.tensor.matmul(
                                pool_ps[:],
                                oh[:, ti * G : (ti + 1) * G],
                                rb3p[:, ti, :],
                                start=(q == 0),
                                stop=(q == TILES - 1),
                            )

            pout = cpool.tile([64, 256], FP, tag="pout")
            if stage == 3:
                nc.vector.tensor_copy(pout[:], pool_ps[:])
            else:
                nc.vector.memset(pout[:], 0.0)
                nc.vector.tensor_add(pout[:, 0:196], pout[:, 0:196], aldst[0:64, 0:196])
            nc.sync.dma_start(d_out[:], pout[:])

    nc.compile()
    return nc


# ----------------------------------------------------------------------------
# Entry point
# ----------------------------------------------------------------------------

def _prepare(inputs):
    key = (
        inputs["edge_index"].tobytes(),
        inputs["batch"].tobytes(),
    )
    kh = hash(key)
    if kh in _cache:
        return _cache[kh]
    edge_index = np.asarray(inputs["edge_index"], np.int64)
    batch = np.asarray(inputs["batch"], np.int64)
    meta = _preprocess(edge_index, batch)
    nc = _build_program(meta)
    _cache[kh] = (meta, nc)
    return meta, nc


def _make_inmaps(inputs, meta):
    x = np.asarray(inputs["x"], np.float32)
    batch = np.asarray(inputs["batch"], np.int64)
    core_nodes = meta["core_nodes"]

    wcats = []
    biases = []
    for l in range(3):
        Wl = np.asarray(inputs[f"W{l}"], np.float64)
        wcats.append(
            _build_wcat(
                Wl,
                np.asarray(inputs[f"a_src{l}"], np.float64),
                np.asarray(inputs[f"a_dst{l}"], np.float64),
            )
        )
        b = np.asarray(inputs[f"b{l}"], np.float32)
        biases.append(np.tile(b[None, :], (128, 1)).astype(np.float32))
    ident = np.eye(128, dtype=BFNP)

    in_maps = []
    for c in range(NCORES):
        nodes = core_nodes[c]
        safe = np.maximum(nodes, 0)
        x0 = x[safe]
        x0[nodes < 0] = 0.0
        # column q*128+p = node (tile q, partition p); core_nodes is tile-major
        x0T = np.ascontiguousarray(x0.T).astype(BFNP)
        in_maps.append(
            {
                "x0T": x0T,
                "wcat0": wcats[0],
                "wcat1": wcats[1],
                "wcat2": wcats[2],
                "bias0": biases[0],
                "bias1": biases[1],
                "bias2": biases[2],
                "ident": ident,
                "idxall": meta["idx_all"][c],
                "maskall": meta["mask_all"][c].astype(BFNP),
                "onehot": meta["onehot"][c].astype(BFNP),
            }
        )
    return in_maps


def _run(inputs, trace=False):
    meta, nc = _prepare(inputs)
    in_maps = _make_inmaps(inputs, meta)
    res = run_bass_kernel_spmd(
        nc, in_maps, core_ids=list(range(NCORES)), trace=trace
    )
    out = np.zeros((G, HD), np.float64)
    for c in range(NCORES):
        out += res.results[c]["pooled"].astype(np.float64)
    return out.astype(np.float32), res


def kernel(**inputs) -> np.ndarray:
    out, _ = _run(inputs, trace=False)
    return out


def kernel_traced(**inputs):
    out, res = _run(inputs, trace=True)
    return out, res
